# revision 3
# baseline (speedup 1.0000x reference)
"""MultiHeadAttention (B=4,T=2048,D=2048,NQ=16,NK=8,H=128) on 8 trn2 cores, v2.

Sharding: core c -> batch b=c//2, half=c%2. Each core computes the partial
output for batch b restricted to q-heads [half*8, half*8+8) (kv-heads
[half*4, half*4+4)); host sums the two partials per batch.

v2: single fused chunk pipeline (proj q/k/v -> attention qi=ch -> o_proj ch),
bf16 everywhere, v projected directly in [token,H] layout (x stationary),
RMS gains folded into weights with a rank-1 g2inv matmul for the sum of
squares broadcast, q kept resident in SBUF (no DRAM spill), softmax
denominators accumulated on DVE in bf16, all ACT functions from the
natural_log_exp_and_others table set (no table switches).
"""
import numpy as np
import ml_dtypes
import concourse.bass as bass
import concourse.tile as tile
from concourse import bacc, mybir
from concourse import bass_utils

B, T, D = 4, 2048, 2048
NQ, NK, H = 16, 8, 128
NH, NKV = 8, 4          # per-core q heads / kv heads
THETA = 10000.0
EPS = 1e-6
TCH = 512               # token chunk
NCH = T // TCH
NDK = D // 128
QCH = 512
NTB = T // 128

f32 = mybir.dt.float32
bf16 = mybir.dt.bfloat16
AF = mybir.ActivationFunctionType

TRACE = False
LAST_EXEC_NS = None
_CACHE = {}


def _install_hook():
    import contextlib, ctypes, sys, types
    if "antenv.axon_hooks" in sys.modules:
        return
    lib = ctypes.CDLL("/opt/axon/libaxon_pjrt.so")
    lib.axon_start_nrt_profile.argtypes = [ctypes.POINTER(ctypes.c_int64), ctypes.c_size_t]
    lib.axon_start_nrt_profile.restype = ctypes.c_int64
    lib.axon_stop_nrt_profile.argtypes = [ctypes.c_char_p]
    lib.axon_stop_nrt_profile.restype = ctypes.c_int64

    @contextlib.contextmanager
    def _hook(output_dir, device_ids):
        import jax
        jax.devices()
        ids = (ctypes.c_int64 * len(device_ids))(*device_ids) if device_ids else None
        rc = lib.axon_start_nrt_profile(ids, len(device_ids) if device_ids else 0)
        if rc != 0:
            raise RuntimeError(f"axon_start_nrt_profile rc={rc}")
        try:
            yield
        finally:
            n = lib.axon_stop_nrt_profile(str(output_dir).encode())
            if n < 0:
                raise RuntimeError(f"axon_stop_nrt_profile rc={n}")

    mod = types.ModuleType("antenv.axon_hooks")
    mod.get_axon_ntff_profile_hook = lambda: _hook
    mod.set_axon_ntff_profile_hook = lambda h: None
    sys.modules["antenv.axon_hooks"] = mod
    bass_utils.upload_artifacts = lambda tmpdir: "local://" + str(tmpdir)


def _build():
    nc = bacc.Bacc("TRN2", target_bir_lowering=False, debug=False, num_devices=8)
    xt_ap = nc.dram_tensor("xt", [D, T], bf16, kind="ExternalInput").ap()
    wq_ap = nc.dram_tensor("wq", [128, NH * NDK * 128], bf16, kind="ExternalInput").ap()
    wk_ap = nc.dram_tensor("wk", [128, NKV * NDK * 128], bf16, kind="ExternalInput").ap()
    wv_ap = nc.dram_tensor("wv", [128, NDK * 512], bf16, kind="ExternalInput").ap()
    wo_ap = nc.dram_tensor("wo", [128, NH * D], bf16, kind="ExternalInput").ap()
    g2q_ap = nc.dram_tensor("g2q", [128, 128], bf16, kind="ExternalInput").ap()
    g2k_ap = nc.dram_tensor("g2k", [128, 128], bf16, kind="ExternalInput").ap()
    cs_ap = nc.dram_tensor("cs", [128, T], bf16, kind="ExternalInput").ap()
    sn_ap = nc.dram_tensor("sn", [128, T], bf16, kind="ExternalInput").ap()
    cm_ap = nc.dram_tensor("cm", [128, 4 * QCH], bf16, kind="ExternalInput").ap()
    oc_ap = nc.dram_tensor("onesc", [128, 1], bf16, kind="ExternalInput").ap()
    or_ap = nc.dram_tensor("onesr", [1, 128], bf16, kind="ExternalInput").ap()
    out_ap = nc.dram_tensor("out", [T, D], bf16, kind="ExternalOutput").ap()

    with tile.TileContext(nc) as tc:
        with tc.tile_pool(name="sb", bufs=1) as sb, \
             tc.tile_pool(name="ps", bufs=1, space="PSUM") as pp:
            xts0 = sb.tile([128, NDK * TCH], bf16, tag="xts", bufs=2, name="xts0")
            wq_t = sb.tile([128, NH * NDK * 128], bf16)
            wk_t = sb.tile([128, NKV * NDK * 128], bf16)
            wv_t = sb.tile([128, NDK * 512], bf16)
            g2q_t = sb.tile([128, 128], bf16)
            g2k_t = sb.tile([128, 128], bf16)
            cm_t = sb.tile([128, 4 * QCH], bf16)
            oc_t = sb.tile([128, 1], bf16)
            or_t = sb.tile([1, 128], bf16)
            eps_t = sb.tile([128, 1], f32)
            nc.vector.memset(eps_t[:], EPS)
            # critical-path-first DMA order: chunk-0 proj needs x dk-slices and
            # the first few q-head weight columns immediately, and the first
            # drain needs g2q right after
            for dk in range(4):
                nc.sync.dma_start(xts0[:, dk * TCH:(dk + 1) * TCH],
                                  xt_ap[dk * 128:(dk + 1) * 128, 0:TCH])
            for hh in range(3):
                nc.sync.dma_start(wq_t[:, hh * NDK * 128:(hh + 1) * NDK * 128],
                                  wq_ap[:, hh * NDK * 128:(hh + 1) * NDK * 128])
            nc.sync.dma_start(g2q_t[:], g2q_ap[:])
            nc.sync.dma_start(g2k_t[:], g2k_ap[:])
            for dk in range(4, NDK):
                nc.sync.dma_start(xts0[:, dk * TCH:(dk + 1) * TCH],
                                  xt_ap[dk * 128:(dk + 1) * 128, 0:TCH])
            for hh in range(3, NH):
                nc.sync.dma_start(wq_t[:, hh * NDK * 128:(hh + 1) * NDK * 128],
                                  wq_ap[:, hh * NDK * 128:(hh + 1) * NDK * 128])
            for kv in range(NKV):
                nc.sync.dma_start(wk_t[:, kv * NDK * 128:(kv + 1) * NDK * 128],
                                  wk_ap[:, kv * NDK * 128:(kv + 1) * NDK * 128])
            nc.sync.dma_start(cm_t[:], cm_ap[:])
            nc.sync.dma_start(oc_t[:], oc_ap[:])
            nc.sync.dma_start(or_t[:], or_ap[:])
            nc.sync.dma_start(wv_t[:], wv_ap[:])

            kT = sb.tile([128, NKV * T], bf16)     # [H, kv*T + t]
            vT = sb.tile([128, NTB * 512], bf16)   # [tok, u*512 + kv*128 + h]

            attn_prev = None  # attn tile of previous chunk, o_proj deferred
            oproj_pending = []

            def oproj_group(pch, ti, dc, attn_tile, tag="ops"):
                trow = pch * 4 + ti
                ops = pp.tile([128, 512], f32, tag=tag, bufs=(1 if tag == "ops" else 2),
                              name=f"ops{pch}_{ti}_{dc}")
                for hh in range(NH):
                    nc.tensor.matmul(
                        ops[:],
                        attn_tile[:, hh * TCH + ti * 128: hh * TCH + (ti + 1) * 128],
                        wod[:, (dc * NH + hh) * 512:(dc * NH + hh + 1) * 512],
                        start=(hh == 0), stop=(hh == NH - 1))
                stg = sb.tile([128, 512], bf16, tag="stg", bufs=2,
                              name=f"stg{pch}_{ti}_{dc}")
                nc.vector.tensor_copy(stg[:], ops[:])
                nc.sync.dma_start(
                    out_ap[trow * 128:(trow + 1) * 128,
                           dc * 512:(dc + 1) * 512], stg[:])

            wod = sb.tile([128, 4 * NH * 512], bf16)  # full wo, [dc][h] major

            for ch in range(NCH):
                t0 = ch * TCH
                if ch == 0:
                    xts = xts0
                else:
                    xts = sb.tile([128, NDK * TCH], bf16, tag="xts", bufs=2,
                                  name=f"xts{ch}")
                    for dk in range(NDK):
                        nc.sync.dma_start(xts[:, dk * TCH:(dk + 1) * TCH],
                                          xt_ap[dk * 128:(dk + 1) * 128, t0:t0 + TCH])
                cs_c = sb.tile([128, TCH], bf16, tag="csc", bufs=1, name=f"cs{ch}")
                nc.sync.dma_start(cs_c[:], cs_ap[:, t0:t0 + TCH])
                sn_c = sb.tile([128, TCH], bf16, tag="snc", bufs=1, name=f"sn{ch}")
                nc.sync.dma_start(sn_c[:], sn_ap[:, t0:t0 + TCH])
                qc = sb.tile([128, NH * TCH], bf16, tag="qcattn", bufs=3,
                             name=f"qc{ch}")

                # ---- q/k projection + RMSNorm + RoPE, [H, tokens] layout ----
                # 4 groups of 3 tiles (tiles 0..7 = q heads, 8..11 = k heads)
                for g in range(4):
                    accs = [pp.tile([128, 512], f32, tag="acc", bufs=3,
                                    name=f"acc{ch}_{g}_{j}") for j in range(3)]
                    for dk in range(NDK):
                        for j in range(3):
                            ti_ = g * 3 + j
                            if ti_ < NH:
                                w_t, hh = wq_t, ti_
                            else:
                                w_t, hh = wk_t, ti_ - NH
                            col = (hh * NDK + dk) * 128
                            nc.tensor.matmul(
                                accs[j][:, 0:TCH], w_t[:, col:col + 128],
                                xts[:, dk * TCH:(dk + 1) * TCH],
                                start=(dk == 0), stop=(dk == NDK - 1))
                    for j in range(3):
                        ti_ = g * 3 + j
                        is_q = ti_ < NH
                        acc = accs[j][:, 0:TCH]
                        sq = sb.tile([128, TCH], bf16, tag="sq", bufs=1,
                                     name=f"sq{ch}_{ti_}")
                        nc.scalar.activation(sq[:], acc, AF.Square)
                        bcs = pp.tile([128, 512], f32, tag="bc", bufs=2,
                                      name=f"bcs{ch}_{ti_}")
                        nc.tensor.matmul(bcs[:, 0:TCH],
                                         (g2q_t if is_q else g2k_t)[:], sq[:],
                                         start=True, stop=True)
                        rstd = sb.tile([128, TCH], f32, tag="rstd", bufs=1,
                                       name=f"rstd{ch}_{ti_}")
                        nc.scalar.activation(rstd[:], bcs[:, 0:TCH], AF.Ln,
                                             bias=eps_t[:], scale=1.0 / H)
                        nc.scalar.activation(rstd[:], rstd[:], AF.Exp, scale=-0.5)
                        qn = sb.tile([128, TCH], bf16, tag="qn", bufs=1,
                                     name=f"qn{ch}_{ti_}")
                        nc.vector.tensor_mul(qn[:], acc, rstd[:])
                        qsw = sb.tile([128, TCH], bf16, tag="qsw", bufs=1,
                                      name=f"qsw{ch}_{ti_}")
                        nc.sync.dma_start(qsw[0:64, :], qn[64:128, :])
                        nc.sync.dma_start(qsw[64:128, :], qn[0:64, :])
                        ta = sb.tile([128, TCH], bf16, tag="ta", bufs=1,
                                     name=f"ta{ch}_{ti_}")
                        nc.vector.tensor_mul(ta[:], qn[:], cs_c[:])
                        tb = sb.tile([128, TCH], bf16, tag="tb", bufs=1,
                                     name=f"tb{ch}_{ti_}")
                        nc.vector.tensor_mul(tb[:], qsw[:], sn_c[:])
                        if is_q:
                            dst = qc[:, ti_ * TCH:(ti_ + 1) * TCH]
                        else:
                            kv = ti_ - NH
                            dst = kT[:, kv * T + t0: kv * T + t0 + TCH]
                        nc.vector.tensor_add(dst, ta[:], tb[:])

                # ---- v projection, [token, H] layout (x stationary) ----
                for ul in range(4):
                    vacc = pp.tile([128, 512], f32, tag="acc", bufs=3,
                                   name=f"vacc{ch}_{ul}")
                    for dk in range(NDK):
                        nc.tensor.matmul(
                            vacc[:, 0:512],
                            xts[:, dk * TCH + ul * 128: dk * TCH + ul * 128 + 128],
                            wv_t[:, dk * 512:(dk + 1) * 512],
                            start=(dk == 0), stop=(dk == NDK - 1))
                    u = ch * 4 + ul
                    nc.scalar.activation(vT[:, u * 512:(u + 1) * 512],
                                         vacc[:, 0:512], AF.Copy)

                if ch == 0:
                    # wo load deferred so it does not compete with the
                    # startup-critical x/wq/wk DMAs; first use is ch1
                    for dc in range(4):
                        for hh in range(NH):
                            nc.sync.dma_start(
                                wod[:, (dc * NH + hh) * 512:(dc * NH + hh + 1) * 512],
                                wo_ap[:, hh * D + dc * 512: hh * D + (dc + 1) * 512])

                # ---- attention for qi = ch, interleaved with o_proj(ch-1) ----
                attn = sb.tile([128, NH * TCH], bf16, tag="qcattn", bufs=3,
                               name=f"attn{ch}")
                nkj = 4 * ch + 4
                for h in range(NH):
                    kv = h // 2
                    o_ps = pp.tile([128, 512], f32, tag="bc", bufs=2,
                                   name=f"o_ps{ch}_{h}")
                    acc_sb = sb.tile([128, QCH], bf16, tag="asb", bufs=1,
                                     name=f"asb{ch}_{h}")

                    def qk(kj):
                        s_ps = pp.tile([128, 512], f32, tag="acc", bufs=3,
                                       name=f"s_ps{ch}_{h}_{kj}")
                        nc.tensor.matmul(
                            s_ps[:],
                            kT[:, kv * T + kj * 128: kv * T + (kj + 1) * 128],
                            qc[:, h * TCH:(h + 1) * TCH], start=True, stop=True)
                        return s_ps

                    sp_cur = qk(0)
                    for kj in range(nkj):
                        sp_next = qk(kj + 1) if kj + 1 < nkj else None
                        pt = sb.tile([128, QCH], bf16, tag="pt", bufs=2,
                                     name=f"pt{ch}_{h}_{kj}")
                        nc.scalar.activation(pt[:], sp_cur[:], AF.Exp, scale=1.0 / H)
                        m = kj - 4 * ch
                        if m >= 0:
                            nc.vector.tensor_mul(pt[:], pt[:],
                                                 cm_t[:, m * QCH:(m + 1) * QCH])
                        if kj == 0:
                            nc.vector.tensor_copy(acc_sb[:], pt[:])
                        else:
                            nc.vector.tensor_add(acc_sb[:], acc_sb[:], pt[:])
                        nc.tensor.matmul(
                            o_ps[:], vT[:, kj * 512 + kv * 128: kj * 512 + (kv + 1) * 128],
                            pt[:], start=(kj == 0), stop=(kj == nkj - 1))
                        sp_cur = sp_next
                        if kj == nkj // 2 and oproj_pending:
                            oproj_group(*oproj_pending.pop(0))
                    den = pp.tile([1, 512], f32, tag="row", bufs=2,
                                  name=f"den{ch}_{h}")
                    nc.tensor.matmul(den[:, 0:QCH], oc_t[:], acc_sb[:],
                                     start=True, stop=True)
                    ld = sb.tile([1, QCH], f32, tag="ld", bufs=1,
                                 name=f"ld{ch}_{h}")
                    nc.scalar.activation(ld[:], den[:, 0:QCH], AF.Ln)
                    rd = sb.tile([1, QCH], bf16, tag="rd", bufs=1,
                                 name=f"rd{ch}_{h}")
                    nc.scalar.activation(rd[:], ld[:], AF.Exp, scale=-1.0)
                    bcd = pp.tile([128, 512], f32, tag="row", bufs=2,
                                  name=f"bcd{ch}_{h}")
                    nc.tensor.matmul(bcd[:, 0:QCH], or_t[:], rd[:],
                                     start=True, stop=True)
                    bcd_sb = sb.tile([128, QCH], f32, tag="bcds", bufs=1,
                                     name=f"bcds{ch}_{h}")
                    nc.vector.tensor_copy(bcd_sb[:], bcd[:, 0:QCH])
                    nc.vector.tensor_mul(attn[:, h * TCH:(h + 1) * TCH],
                                         o_ps[:], bcd_sb[:])
                    # fill exp-wait gaps with o_proj matmuls of previous chunk
                    if oproj_pending:
                        oproj_group(*oproj_pending.pop(0))
                while oproj_pending:
                    oproj_group(*oproj_pending.pop(0))
                attn_prev = attn
                oproj_pending = [(ch, ti, dc, attn) for dc in range(4)
                                 for ti in range(4)]

            # drain last chunk's o_proj, alternating psum banks ("bc" is
            # idle after the last normalize) so the groups pipeline
            drain_i = 0
            while oproj_pending:
                oproj_group(*oproj_pending.pop(0),
                            tag=("ops" if drain_i % 2 == 0 else "bc"))
                drain_i += 1

    import concourse.bacc as _bacc_mod
    _orig_tables = _bacc_mod.get_activation_tables

    def _combined_only(arch):
        t = _orig_tables(arch)
        name = "natural_log_exp_and_others"
        if name in t and all(f in t[name] for f in
                             (AF.Exp, AF.Ln, AF.Square, AF.Copy)):
            # keep every entry (ids are positional indexes into
            # act_info.json) but leave only the combined set non-empty so
            # the load-insertion pass always picks it
            return {k: (v if k == name else set()) for k, v in t.items()}
        return t

    _bacc_mod.get_activation_tables = _combined_only
    try:
        nc.compile()
    finally:
        _bacc_mod.get_activation_tables = _orig_tables
    return nc


def _numpy_ref(x, mask, position, qp, kvp, op, qns, kns):
    def rms(v, s):
        var = (v * v).mean(-1, keepdims=True)
        return v / np.sqrt(var + EPS) * (1.0 + s)

    def rope(v, pos):
        ts = THETA ** (np.arange(64, dtype=np.float32) * 2.0 / H)
        ang = pos.astype(np.float32)[:, :, None, None] / ts
        sn, cs = np.sin(ang), np.cos(ang)
        x1, x2 = v[..., :64], v[..., 64:]
        return np.concatenate([x1 * cs - x2 * sn, x2 * cs + x1 * sn], -1)

    q = np.einsum('BTD,NDH->BTNH', x, qp)
    k = np.einsum('BTD,KDH->BTKH', x, kvp[0])
    v = np.einsum('BTD,KDH->BTKH', x, kvp[1])
    q = rope(rms(q, qns), position) * (H ** -0.5)
    k = rope(rms(k, kns), position)
    q = q.transpose(0, 2, 1, 3)
    k = np.repeat(k.transpose(0, 2, 1, 3), NQ // NK, 1)
    v = np.repeat(v.transpose(0, 2, 1, 3), NQ // NK, 1)
    s = np.einsum('BHtD,BHTD->BHtT', q, k) / np.sqrt(np.float32(H))
    s = np.where(mask[:, None], s, np.float32(-2.3819763e+38))
    s = s - s.max(-1, keepdims=True)
    w = np.exp(s)
    w /= w.sum(-1, keepdims=True)
    o = np.einsum('BHtT,BHTD->BHtD', w, v)
    return np.einsum('BNTH,NHD->BTD', o, op).astype(np.float32)


def _pack_qk(w, gain):
    """(nh, D, H) -> (128, nh*NDK*128) bf16 with per-H gain folded in."""
    nh = w.shape[0]
    wg = w * gain[None, None, :]
    a = wg.reshape(nh, NDK, 128, H).transpose(2, 0, 1, 3)
    return np.ascontiguousarray(a.reshape(128, nh * NDK * H)).astype(ml_dtypes.bfloat16)


def kernel(**inputs):
    global LAST_EXEC_NS
    x = np.asarray(inputs["x"], np.float32)
    mask = np.asarray(inputs["mask"])
    position = np.asarray(inputs["position"])
    qp = np.asarray(inputs["q_proj"], np.float32)
    kvp = np.asarray(inputs["kv_proj"], np.float32)
    op = np.asarray(inputs["o_proj"], np.float32)
    qns = np.asarray(inputs["q_norm_scale"], np.float32)
    kns = np.asarray(inputs["k_norm_scale"], np.float32)

    tril = np.tril(np.ones((T, T), bool))
    if mask.shape != (B, T, T) or not all(np.array_equal(mask[b], tril) for b in range(B)):
        return _numpy_ref(x, mask, position, qp, kvp, op, qns, kns)

    if "nc" not in _CACHE:
        _CACHE["nc"] = _build()
    nc = _CACHE["nc"]

    gq = 1.0 + qns
    gk = 1.0 + kns
    g2q = np.broadcast_to((1.0 / (gq * gq))[:, None], (128, 128))
    g2k = np.broadcast_to((1.0 / (gk * gk))[:, None], (128, 128))
    g2q = np.ascontiguousarray(g2q).astype(ml_dtypes.bfloat16)
    g2k = np.ascontiguousarray(g2k).astype(ml_dtypes.bfloat16)

    halves = []
    for half in range(2):
        wq = _pack_qk(qp[half * NH:(half + 1) * NH], gq)
        wk = _pack_qk(kvp[0, half * NKV:(half + 1) * NKV], gk)
        # wv: [128, dk*512 + kv*128 + h] = Wv[kv, dk*128+p, h]
        wvs = kvp[1, half * NKV:(half + 1) * NKV]             # (NKV, D, H)
        a = wvs.reshape(NKV, NDK, 128, H).transpose(2, 1, 0, 3)  # (p, dk, kv, h)
        wv = np.ascontiguousarray(a.reshape(128, NDK * NKV * H)).astype(ml_dtypes.bfloat16)
        wo = np.ascontiguousarray(
            op[half * NH:(half + 1) * NH].transpose(1, 0, 2).reshape(128, NH * D)
        ).astype(ml_dtypes.bfloat16)
        halves.append((wq, wk, wv, wo))

    ts = THETA ** (np.arange(64, dtype=np.float64) * 2.0 / H)
    fidx = np.arange(QCH)[None, :]
    pidx = np.arange(128)[:, None]
    cm = np.concatenate(
        [(fidx >= m * 128 + pidx).astype(np.float32) for m in range(4)], axis=1)
    cm = np.ascontiguousarray(cm).astype(ml_dtypes.bfloat16)

    in_maps = []
    for c in range(8):
        b, half = c // 2, c % 2
        wq, wk, wv, wo = halves[half]
        ang = position[b].astype(np.float64)[None, :] / ts[:, None]
        sn = np.sin(ang).astype(np.float32)
        cs = np.cos(ang).astype(np.float32)
        in_maps.append({
            "xt": np.ascontiguousarray(x[b].T).astype(ml_dtypes.bfloat16),
            "wq": wq, "wk": wk, "wv": wv, "wo": wo,
            "g2q": g2q, "g2k": g2k,
            "cs": np.ascontiguousarray(np.concatenate([cs, cs], 0)).astype(ml_dtypes.bfloat16),
            "sn": np.ascontiguousarray(np.concatenate([-sn, sn], 0)).astype(ml_dtypes.bfloat16),
            "cm": cm,
            "onesc": np.ones((128, 1), ml_dtypes.bfloat16),
            "onesr": np.ones((1, 128), ml_dtypes.bfloat16),
        })

    if TRACE:
        _install_hook()
    last_err = None
    for _ in range(3):
        try:
            res = bass_utils.run_bass_kernel_spmd(nc, in_maps, list(range(8)), trace=TRACE)
            break
        except Exception as e:  # transient NRT device wedge
            last_err = e
    else:
        raise last_err
    LAST_EXEC_NS = getattr(res, "exec_time_ns", None)

    out = np.empty((B, T, D), np.float32)
    for b in range(B):
        out[b] = (res.results[2 * b]["out"].astype(np.float32)
                  + res.results[2 * b + 1]["out"].astype(np.float32))
    return out
